# revision 1
# baseline (speedup 1.0000x reference)
"""ExpSyn kernel: diagonal linear recurrence isyn_t = beta*isyn_{t-1} + x_t.

The DVE tensor_tensor_scan runs at ~2.12 ns/col regardless of dtype (serial
dependency), so a plain per-row scan is DVE-bound at ~70us/core. This kernel
halves the DVE scan length with a depth-1 odd-even decomposition:

  pack:   x'_i   = beta * x_{2i} + x_{2i+1}      (ACT mult + GPSIMD add)
  scan:   y_odd  = scan(x', beta^2)              (DVE, T/2 cols)
  unpack: y_{2i} = beta * y_odd_{i-1} + x_{2i}   (ACT mult + DVE add)

Host lays x out de-interleaved (evens in cols 0:T/2, odds in T/2:T) so every
op is a full-width stride-1 2D AP. Cross-engine latency is hidden by TWO
software-pipeline lags: the load+pack front runs 2 blocks ahead, and the
unpack add for block k is issued AFTER block k+1's scan, so the DVE never
waits on ACT's unpack multiply (the next scan is the bubble filler). The odd
half of each block stores immediately after its scan (it is final then); the
even half stores after the unpack add. Block 0 keeps the simple chunked
plain-scan so the pipeline head stays short.

I/O is fp16 (halves HBM traffic; the scan carries fp32 state internally;
rel err ~1.1e-3 vs the 2e-2 gate). Loads ride the sync ring, stores the
GPSIMD ring, ACT/DVE/GPSIMD compute.
"""

import numpy as np

DT = 1e-4
B, T, N = 16, 4096, 512
NCORES = 8
BLOC = B // NCORES          # 2 batches per core
ROWS = BLOC * N             # 1024 scan rows per core
NG = N // 128               # 4 channel groups of 128
NTILES = ROWS // 128        # 8 row-blocks per core
H = T // 2                  # 2048
NPOW = 2                    # beta, beta^2

_cached = None


def _build():
    import concourse.bacc as bacc
    import concourse.mybir as mybir
    from concourse import tile

    nc = bacc.Bacc("TRN2", debug=False, num_devices=NCORES)
    f32 = mybir.dt.float32
    f16 = mybir.dt.float16
    mult, add = mybir.AluOpType.mult, mybir.AluOpType.add

    x = nc.dram_tensor("x", [ROWS, T], f16, kind="ExternalInput")
    beta_d = nc.dram_tensor("beta", [128, NG * NPOW], f32, kind="ExternalInput")
    y = nc.dram_tensor("y", [ROWS, T], f16, kind="ExternalOutput")

    with tile.TileContext(nc) as tc:
        with (
            tc.tile_pool(name="const", bufs=1) as cpool,
            tc.tile_pool(name="work", bufs=4) as wpool,
            tc.tile_pool(name="b0", bufs=1) as b0pool,
        ):
            bsb = cpool.tile([128, NG * NPOW], f32, name="bsb")
            nc.sync.dma_start(out=bsb[:, :], in_=beta_d[:, :])

            def pw(g, j):            # [128,1] scalar: beta^(2^j) for group g
                return bsb[:, g * NPOW + j:g * NPOW + j + 1]

            def pwb(g, j, n):        # broadcast for the scan
                return pw(g, j).broadcast_to([128, n])

            # ---- blocks 0,1: chunked plain scans (fill the DVE while the
            # tree pipeline warms); their stores ride the sync ring, which is
            # idle after the loads, so no compute queue is head-of-line
            # blocked by a store waiting on a scan ----
            bounds = [0, 128, 512, 1536, T]
            xt0 = b0pool.tile([128, T], f16, name="xt0")
            for c in range(len(bounds) - 1):
                lo, hi = bounds[c], bounds[c + 1]
                nc.sync.dma_start(out=xt0[:, lo:hi], in_=x[0:128, lo:hi])
            for c in range(len(bounds) - 1):
                lo, hi = bounds[c], bounds[c + 1]
                init = 0.0 if c == 0 else xt0[:, lo - 1:lo]
                nc.vector.tensor_tensor_scan(
                    xt0[:, lo:hi], pwb(0, 0, hi - lo), xt0[:, lo:hi],
                    init, mult, add)
            nc.gpsimd.dma_start(out=y[0:128, :], in_=xt0[:, :])

            xt1 = b0pool.tile([128, T], f16, name="xt1")
            nc.sync.dma_start(out=xt1[:, :], in_=x[128:256, :])
            nc.vector.tensor_tensor_scan(
                xt1[:, 0:H], pwb(1, 0, H), xt1[:, 0:H], 0.0, mult, add)
            nc.gpsimd.dma_start(out=y[128:256, 0:H], in_=xt1[:, 0:H])
            nc.vector.tensor_tensor_scan(
                xt1[:, H:T], pwb(1, 0, H), xt1[:, H:T],
                xt1[:, H - 1:H], mult, add)
            nc.gpsimd.dma_start(out=y[128:256, H:T], in_=xt1[:, H:T])

            # ---- blocks 2..7: depth-1 odd-even, double software-pipelined --
            tiles = {}

            def front(k):
                g = k % NG
                r0 = k * 128
                X0 = wpool.tile([128, T], f16, tag="x0", name=f"x0_{k}")
                TM1 = wpool.tile([128, H], f16, tag="tm1", name=f"tm1_{k}")
                X1 = wpool.tile([128, H], f16, tag="x1", name=f"x1_{k}")
                tiles[k] = (X0, X1)
                nc.sync.dma_start(out=X0[:, :], in_=x[r0:r0 + 128, :])
                # pack: TM1 = beta * x_even ; X1 = TM1 + x_odd
                nc.scalar.mul(TM1[:, :], X0[:, 0:H], pw(g, 0))
                nc.vector.tensor_tensor(
                    out=X1[:, :], in0=TM1[:, :], in1=X0[:, H:T], op=add)

            def unpack_add(k):
                X0, Y, TE = tiles.pop((k, "u"))
                r0 = k * 128
                nc.vector.tensor_tensor(
                    out=Y[:, 0:H], in0=TE[:, :], in1=X0[:, 0:H], op=add)
                nc.gpsimd.dma_start(out=y[r0:r0 + 128, 0:H], in_=Y[:, 0:H])

            front(2)
            front(3)
            for k in range(2, NTILES - 1):
                g = k % NG
                r0 = k * 128
                X0, X1 = tiles.pop(k)
                Y = wpool.tile([128, T], f16, tag="y", name=f"y_{k}")
                TE = wpool.tile([128, H], f16, tag="te", name=f"te_{k}")

                # odd outputs: scan of the packed stream -> Y[:, H:T]
                nc.vector.memset(Y[:, H - 1:H], 0.0)
                nc.vector.tensor_tensor_scan(
                    Y[:, H:T], pwb(g, 1, H), X1[:, :], 0.0, mult, add)
                # front first: TM1(k+2) must not queue behind TE(k) on ACT
                # (TE waits on scan(k); TM1 only needs an already-loaded X0)
                if k + 2 < NTILES:
                    front(k + 2)
                nc.gpsimd.dma_start(out=y[r0:r0 + 128, H:T], in_=Y[:, H:T])
                # unpack multiply on ACT; the add is deferred one block
                nc.scalar.mul(TE[:, :], Y[:, H - 1:T - 1], pw(g, 0))
                tiles[(k, "u")] = (X0, Y, TE)

                if k > 2:
                    unpack_add(k - 1)

            # ---- last block: halves, so the tail TE/tt/store chain overlaps
            k = NTILES - 1
            g = k % NG
            r0 = k * 128
            X0, X1 = tiles.pop(k)
            Y = wpool.tile([128, T], f16, tag="y", name=f"y_{k}")
            TE = wpool.tile([128, H], f16, tag="te", name=f"te_{k}")
            Q = H // 2
            nc.vector.memset(Y[:, H - 1:H], 0.0)
            nc.vector.tensor_tensor_scan(
                Y[:, H:H + Q], pwb(g, 1, Q), X1[:, 0:Q], 0.0, mult, add)
            nc.gpsimd.dma_start(out=y[r0:r0 + 128, H:H + Q], in_=Y[:, H:H + Q])
            nc.scalar.mul(TE[:, 0:Q], Y[:, H - 1:H + Q - 1], pw(g, 0))
            nc.vector.tensor_tensor_scan(
                Y[:, H + Q:T], pwb(g, 1, H - Q), X1[:, Q:H],
                Y[:, H + Q - 1:H + Q], mult, add)
            nc.gpsimd.dma_start(out=y[r0:r0 + 128, H + Q:T], in_=Y[:, H + Q:T])
            nc.scalar.mul(TE[:, Q:H], Y[:, H + Q - 1:T - 1], pw(g, 0))
            unpack_add(k - 1)
            nc.vector.tensor_tensor(
                out=Y[:, 0:Q], in0=TE[:, 0:Q], in1=X0[:, 0:Q], op=add)
            nc.gpsimd.dma_start(out=y[r0:r0 + 128, 0:Q], in_=Y[:, 0:Q])
            nc.vector.tensor_tensor(
                out=Y[:, Q:H], in0=TE[:, Q:H], in1=X0[:, Q:H], op=add)
            nc.gpsimd.dma_start(out=y[r0:r0 + 128, Q:H], in_=Y[:, Q:H])

    nc.compile()
    return nc


def _get_nc():
    global _cached
    if _cached is None:
        _cached = _build()
    return _cached


def _perm():
    t = np.arange(T)
    return (t % 2) * H + t // 2     # device col for time t


def _make_in_maps(data, tau_syn):
    tau = np.asarray(tau_syn, dtype=np.float64)
    beta = np.exp(-DT / tau)  # (1, N) f64
    bt = np.empty((128, NG * NPOW), dtype=np.float32)
    for g in range(NG):
        for j in range(NPOW):
            bt[:, g * NPOW + j] = (beta[0, g * 128:(g + 1) * 128] ** (1 << j)
                                   ).astype(np.float32)
    xt = np.asarray(data, dtype=np.float32).transpose(0, 2, 1).astype(np.float16)
    xt = np.ascontiguousarray(xt).reshape(NCORES, ROWS, T)
    perm = _perm()
    xs = xt.copy()
    xs[:, 256:, perm] = xt[:, 256:, :]
    return [{"x": xs[c], "beta": bt} for c in range(NCORES)]


def kernel(data, tau_syn):
    from concourse.bass_utils import run_bass_kernel_spmd

    nc = _get_nc()
    in_maps = _make_in_maps(data, tau_syn)
    res = run_bass_kernel_spmd(nc, in_maps, list(range(NCORES)))
    out = np.stack([res.results[c]["y"] for c in range(NCORES)])  # (8, ROWS, T)
    perm = _perm()
    out[:, 256:, :] = out[:, 256:, perm]
    out = out.astype(np.float32).reshape(B, N, T).transpose(0, 2, 1)
    return np.ascontiguousarray(out)

